# revision 13
# baseline (speedup 1.0000x reference)
"""Trainium2 Bass kernel for nn_fqconv_ansatz_layer.

Math: the 12-qubit circuit factorizes into 4 independent 3-qubit circuits
(triples {k, 4+k, 8+k}): the initial RX layer is a product state and every
controlled gate pairs wires within one triple. Per circuit and triple:
  ev3 = S3 @ |W_k' m|^2,  m = kron(u_k, u_{4+k}, u_{8+k}),  u_w = (cos th/2, sin th/2)
with W_k' = (product of the 6 controlled 8x8 gates) @ diag((-i)^popcount) a
weights-only 8x8 complex matrix precomputed on host.

Device kernel (per core, 256 circuits, raw Bass with explicit semaphores —
the CoreV3 ISA allows only one embedded sync wait per instruction, so waits
are standalone): cos/sin via Sin activations (quarter/half-angle keeps the
operand inside the valid [-pi, pi] range), the kron via 2 broadcast vector
multiplies, one PE transpose, then blockdiag matmuls + squares + a signed
marginal matmul pair accumulating in PSUM.
"""

import os
import numpy as np

N_Q = 12
GATES = [
    ('z', 0, 3, 11), ('x', 1, 2, 10), ('z', 2, 1, 9), ('x', 3, 0, 8),
    ('x', 4, 3, 11), ('z', 5, 2, 10), ('x', 6, 1, 9), ('z', 7, 0, 8),
    ('z', 8, 11, 7), ('x', 9, 10, 6), ('z', 10, 9, 5), ('x', 11, 8, 4),
    ('x', 12, 11, 7), ('z', 13, 10, 6), ('x', 14, 9, 5), ('z', 15, 8, 4),
    ('z', 16, 7, 3), ('x', 17, 6, 2), ('z', 18, 5, 1), ('x', 19, 4, 0),
    ('x', 20, 7, 3), ('z', 21, 6, 2), ('x', 22, 5, 1), ('z', 23, 4, 0),
]

N_CORES = 8
B = 512
G = 4
N_CIRC = B * G            # 2048
PER_CORE = N_CIRC // N_CORES  # 256

# consts layout: [128, 280] f32
#   rows 0:64,  cols   0: 64  -> AT  (lhsT of Re(W') matmul, block-diag over (c2, k))
#   rows 0:64,  cols  64:128  -> BT  (lhsT of Im(W') matmul)
#   rows 0:64,  cols 128:152  -> ST  (lhsT of signed marginal sum, [64, 24])
#   rows 0:128, cols 152:280 -> identity 128x128 (PE transpose operand)
CONST_COLS = 280


def _rx2(theta):
    t = theta * 0.5
    return np.array([[np.cos(t), -1j * np.sin(t)],
                     [-1j * np.sin(t), np.cos(t)]], np.complex128)


def _rz2(theta):
    t = theta * 0.5
    return np.array([[np.exp(-1j * t), 0.0], [0.0, np.exp(1j * t)]], np.complex128)


def _ctrl_gate_3q(U2, pc, pt):
    M = np.zeros((8, 8), np.complex128)
    for i in range(8):
        for j in range(8):
            ic, jc = (i >> pc) & 1, (j >> pc) & 1
            it, jt = (i >> pt) & 1, (j >> pt) & 1
            mask = ~((1 << pc) | (1 << pt)) & 7
            if (i & mask) != (j & mask) or ic != jc:
                continue
            M[i, j] = (U2[it, jt] if ic == 1 else (1.0 if it == jt else 0.0))
    return M


def _build_consts(weights):
    """weights [24] f32 -> consts [128, 280] f32 (shared by all cores)."""
    w = np.asarray(weights, np.float64)
    popc = np.array([bin(j).count('1') for j in range(8)])
    A = np.zeros((4, 8, 8))
    Bm = np.zeros((4, 8, 8))
    for k in range(4):
        bitpos = {k: 2, 4 + k: 1, 8 + k: 0}
        W = np.eye(8, dtype=np.complex128)
        for kind, wi, c, t in GATES:
            if c % 4 != k:
                continue
            U2 = _rz2(w[wi]) if kind == 'z' else _rx2(w[wi])
            W = _ctrl_gate_3q(U2, bitpos[c], bitpos[t]) @ W
        Wp = W @ np.diag((-1j) ** popc)
        A[k] = Wp.real
        Bm[k] = Wp.imag
    S3 = np.zeros((3, 8))
    for l in range(3):
        for j in range(8):
            S3[l, j] = 1.0 if ((j >> (2 - l)) & 1) == 0 else -1.0

    consts = np.zeros((128, CONST_COLS), np.float32)
    for c2 in range(2):
        for k in range(4):
            r = c2 * 32 + k * 8
            # lhsT[j_in, j_out] = M[j_out, j_in]
            consts[r:r + 8, r:r + 8] = A[k].T
            consts[r:r + 8, 64 + r:64 + r + 8] = Bm[k].T
            for l, wire in enumerate((k, 4 + k, 8 + k)):
                consts[r:r + 8, 128 + c2 * 12 + wire] = S3[l]
    consts[:, 152:280] = np.eye(128, dtype=np.float32)
    return consts


def _build_program():
    import concourse.bass as bass
    from concourse import mybir
    from contextlib import ExitStack

    f32 = mybir.dt.float32
    AF = mybir.ActivationFunctionType
    nc = bass.Bass()
    ang = nc.declare_dram_parameter("ang", [PER_CORE, 12], f32, isOutput=False)
    consts = nc.declare_dram_parameter("consts", [128, CONST_COLS], f32, isOutput=False)
    evout = nc.declare_dram_parameter("evout", [24, 128], f32, isOutput=True)

    with ExitStack() as ctx:
        e = ctx.enter_context
        const_sb = e(nc.sbuf_tensor([128, CONST_COLS], f32))
        ang_t = e(nc.sbuf_tensor([128, 24], f32))
        u = e(nc.sbuf_tensor([128, 24], f32))
        usq = e(nc.sbuf_tensor([128, 24], f32))
        cs = e(nc.sbuf_tensor([128, 48], f32))   # col = c2*24 + j2*12 + w
        z4 = e(nc.sbuf_tensor([128, 32], f32))   # col = c2*16 + k*4 + j2*2 + j1
        m = e(nc.sbuf_tensor([128, 64], f32))    # col = c2*32 + k*8 + j2*4 + j1*2 + j0
        mT_sb = e(nc.sbuf_tensor([64, 128], f32))
        sq = e(nc.sbuf_tensor([64, 256], f32))
        ev_sb = e(nc.sbuf_tensor([24, 128], f32))
        mT_ps = e(nc.psum_tensor([64, 128], f32))
        reim_ps = e(nc.psum_tensor([64, 256], f32))
        ev_ps = e(nc.psum_tensor([24, 128], f32))
        s_dma = e(nc.semaphore("s_dma"))
        s_act = e(nc.semaphore("s_act"))
        s_pe = e(nc.semaphore("s_pe"))
        s_dve = e(nc.semaphore("s_dve"))
        block = e(nc.Block())

        @block.sync
        def _(sync):
            sync.dma_start(
                out=const_sb[:], in_=consts[:, :]).then_inc(s_dma, 16)
            sync.dma_start(
                out=ang_t[:],
                in_=ang[:, :].rearrange("(p c) w -> p (c w)", c=2),
            ).then_inc(s_dma, 16)
            sync.wait_ge(s_act, 7)
            sync.dma_start(out=evout[:, :], in_=ev_sb[:]).then_inc(s_dma, 16)
            sync.wait_ge(s_dma, 48)

        @block.scalar
        def _(scalar):
            cv = cs[:].rearrange("p (c j w) -> p c j w", c=2, j=2, w=12)
            a2 = ang_t[:].rearrange("p (c w) -> p c w", c=2)
            scalar.wait_ge(s_dma, 32)
            # u = sin(x/4); cos(x/2) = 1 - 2u^2; sin(x/2) = Sin(0.5 x)
            # (|x| < 2pi for N(0,1) angles, so all Sin operands are in-range)
            nc.scalar.activation(u[:], ang_t[:], AF.Sin, scale=0.25).then_inc(s_act, 1)
            scalar.wait_ge(s_act, 1)
            nc.scalar.activation(usq[:], u[:], AF.Square).then_inc(s_act, 1)
            scalar.wait_ge(s_act, 2)
            nc.scalar.activation(
                cv[:, :, 0, :], usq[:].rearrange("p (c w) -> p c w", c=2),
                AF.Identity, bias=1.0, scale=-2.0).then_inc(s_act, 1)
            nc.scalar.activation(cv[:, :, 1, :], a2, AF.Sin, scale=0.5).then_inc(s_act, 1)
            scalar.wait_ge(s_pe, 1)
            nc.scalar.copy(mT_sb[:], mT_ps[:]).then_inc(s_act, 1)
            scalar.wait_ge(s_pe, 3)
            nc.scalar.activation(sq[:], reim_ps[:], AF.Square).then_inc(s_act, 1)
            scalar.wait_ge(s_pe, 5)
            nc.scalar.copy(ev_sb[:], ev_ps[:]).then_inc(s_act, 1)

        @block.vector
        def _(vector):
            # kron build: z4[(k,j2,j1)] = cs[j2*12+k] * cs[j1*12+4+k]
            #             m[(k,j2,j1,j0)] = z4[(k,j2,j1)] * cs[j0*12+8+k]
            csw = cs[:].rearrange("p (c j w) -> p c w j", c=2, j=2, w=12)
            z4v = z4[:].rearrange("p (c k a b) -> p c k a b", c=2, k=4, a=2, b=2)
            mv = m[:].rearrange("p (c k a b) -> p c k a b", c=2, k=4, a=4, b=2)
            vector.wait_ge(s_act, 4)
            for c2 in range(2):
                in0 = csw[:, c2, 0:4, :]                             # [p, k, j2]
                in0 = in0.unsqueeze(3).to_broadcast((128, 4, 2, 2))  # bcast j1
                in1 = csw[:, c2, 4:8, :]                             # [p, k, j1]
                in1 = in1.unsqueeze(2).to_broadcast((128, 4, 2, 2))  # bcast j2
                nc.vector.tensor_mul(z4v[:, c2], in0, in1).then_inc(s_dve, 1)
            vector.wait_ge(s_dve, 2)
            for c2 in range(2):
                in0 = z4v[:, c2].rearrange("p k a b -> p k (a b)")   # [p, 4, 4]
                in0 = in0.unsqueeze(3).to_broadcast((128, 4, 4, 2))  # bcast j0
                in1 = csw[:, c2, 8:12, :]                            # [p, k, j0]
                in1 = in1.unsqueeze(2).to_broadcast((128, 4, 4, 2))  # bcast j2 j1
                nc.vector.tensor_mul(mv[:, c2], in0, in1).then_inc(s_dve, 1)

        @block.tensor
        def _(tensor):
            tensor.wait_ge(s_dma, 32)
            tensor.wait_ge(s_dve, 4)
            nc.tensor.transpose(mT_ps[:], m[:], const_sb[:, 152:280]).then_inc(s_pe, 1)
            tensor.wait_ge(s_act, 5)
            nc.tensor.matmul(reim_ps[:, 0:128], const_sb[0:64, 0:64], mT_sb[:],
                             start=True, stop=True).then_inc(s_pe, 1)
            nc.tensor.matmul(reim_ps[:, 128:256], const_sb[0:64, 64:128], mT_sb[:],
                             start=True, stop=True).then_inc(s_pe, 1)
            tensor.wait_ge(s_act, 6)
            # ev = ST^T (sq_re + sq_im): two matmuls accumulating in PSUM
            nc.tensor.matmul(ev_ps[:], const_sb[0:64, 128:152], sq[:, 0:128],
                             start=True, stop=False).then_inc(s_pe, 1)
            nc.tensor.matmul(ev_ps[:], const_sb[0:64, 128:152], sq[:, 128:256],
                             start=False, stop=True).then_inc(s_pe, 1)

    return nc


_CACHE = {}
LAST_RESULTS = None


def _get_program():
    if "nc" not in _CACHE:
        _CACHE["nc"] = _build_program()
    return _CACHE["nc"]


def kernel(x, fqconv_weights):
    global LAST_RESULTS
    from concourse.bass_utils import run_bass_kernel_spmd

    x = np.ascontiguousarray(np.asarray(x, np.float32))
    consts = _build_consts(fqconv_weights)
    nc = _get_program()

    in_maps = []
    for i in range(N_CORES):
        g, b0 = i // 2, (i % 2) * PER_CORE
        ang_i = np.ascontiguousarray(
            x[b0:b0 + PER_CORE, 3 * g:3 * g + 3].reshape(PER_CORE, 12))
        in_maps.append({"ang": ang_i, "consts": consts})

    res = run_bass_kernel_spmd(
        nc, in_maps, list(range(N_CORES)),
        trace=bool(int(os.environ.get("FQ_TRACE", "0"))),
    )
    LAST_RESULTS = res

    ev_all = np.empty((N_CIRC, 12), np.float32)
    for i in range(N_CORES):
        t = res.results[i]["evout"].reshape(2, 12, 128)
        ev_all[i * PER_CORE:(i + 1) * PER_CORE] = (
            t.transpose(2, 0, 1).reshape(PER_CORE, 12))
    return ev_all.reshape(-1).reshape(B, 12, 2, 2).astype(np.float32)


# revision 25
# speedup vs baseline: 1.0987x; 1.0987x over previous
"""Trainium2 Bass kernel for nn_fqconv_ansatz_layer.

Math: the 12-qubit circuit factorizes into 4 independent 3-qubit circuits
(triples {k, 4+k, 8+k}): the initial RX layer is a product state and every
controlled gate pairs wires within one triple. Per circuit and triple:
  ev3 = S3 @ |W_k' m|^2,  m = kron(u_k, u_{4+k}, u_{8+k}),  u_w = (cos th/2, sin th/2)
with W_k' = (product of the 6 controlled 8x8 gates) @ diag((-i)^popcount) a
weights-only 8x8 complex matrix precomputed on host.

Device kernel (per core, 256 circuits, raw Bass with explicit semaphores —
the CoreV3 ISA allows only one embedded sync wait per instruction, so waits
are standalone): cos/sin via Sin activations (quarter/half-angle keeps the
operand inside the valid [-pi, pi] range), the kron via 2 broadcast vector
multiplies, one PE transpose, then blockdiag matmuls + squares + a signed
marginal matmul pair accumulating in PSUM.
"""

import os
import numpy as np

N_Q = 12
GATES = [
    ('z', 0, 3, 11), ('x', 1, 2, 10), ('z', 2, 1, 9), ('x', 3, 0, 8),
    ('x', 4, 3, 11), ('z', 5, 2, 10), ('x', 6, 1, 9), ('z', 7, 0, 8),
    ('z', 8, 11, 7), ('x', 9, 10, 6), ('z', 10, 9, 5), ('x', 11, 8, 4),
    ('x', 12, 11, 7), ('z', 13, 10, 6), ('x', 14, 9, 5), ('z', 15, 8, 4),
    ('z', 16, 7, 3), ('x', 17, 6, 2), ('z', 18, 5, 1), ('x', 19, 4, 0),
    ('x', 20, 7, 3), ('z', 21, 6, 2), ('x', 22, 5, 1), ('z', 23, 4, 0),
]

N_CORES = 8
B = 512
G = 4
N_CIRC = B * G            # 2048
PER_CORE = N_CIRC // N_CORES  # 256

# consts layout: [128, 154] f32
#   rows 0:64,  cols   0: 64  -> AT  (lhsT of Re(W') matmul, block-diag over (c2, k))
#   rows 0:64,  cols  64:128  -> BT  (lhsT of Im(W') matmul)
#   rows 0:64,  cols 128:152  -> ST  (lhsT of signed marginal sum, [64, 24])
#   col 152 -> zeros, col 153 -> ones (activation bias operands)
CONST_COLS = 154
USE_F32R = False  # fp32r needs f32r-rounded producers; not worth it here


def _rx2(theta):
    t = theta * 0.5
    return np.array([[np.cos(t), -1j * np.sin(t)],
                     [-1j * np.sin(t), np.cos(t)]], np.complex128)


def _rz2(theta):
    t = theta * 0.5
    return np.array([[np.exp(-1j * t), 0.0], [0.0, np.exp(1j * t)]], np.complex128)


def _ctrl_gate_3q(U2, pc, pt):
    M = np.zeros((8, 8), np.complex128)
    for i in range(8):
        for j in range(8):
            ic, jc = (i >> pc) & 1, (j >> pc) & 1
            it, jt = (i >> pt) & 1, (j >> pt) & 1
            mask = ~((1 << pc) | (1 << pt)) & 7
            if (i & mask) != (j & mask) or ic != jc:
                continue
            M[i, j] = (U2[it, jt] if ic == 1 else (1.0 if it == jt else 0.0))
    return M


def _build_consts(weights):
    """weights [24] f32 -> consts [128, 280] f32 (shared by all cores)."""
    w = np.asarray(weights, np.float64)
    popc = np.array([bin(j).count('1') for j in range(8)])
    A = np.zeros((4, 8, 8))
    Bm = np.zeros((4, 8, 8))
    for k in range(4):
        bitpos = {k: 2, 4 + k: 1, 8 + k: 0}
        W = np.eye(8, dtype=np.complex128)
        for kind, wi, c, t in GATES:
            if c % 4 != k:
                continue
            U2 = _rz2(w[wi]) if kind == 'z' else _rx2(w[wi])
            W = _ctrl_gate_3q(U2, bitpos[c], bitpos[t]) @ W
        Wp = W @ np.diag((-1j) ** popc)
        A[k] = Wp.real
        Bm[k] = Wp.imag
    S3 = np.zeros((3, 8))
    for l in range(3):
        for j in range(8):
            S3[l, j] = 1.0 if ((j >> (2 - l)) & 1) == 0 else -1.0

    consts = np.zeros((128, CONST_COLS), np.float32)
    for c2 in range(2):
        for k in range(4):
            r = c2 * 32 + k * 8
            # lhsT[j_in, j_out] = M[j_out, j_in]
            consts[r:r + 8, r:r + 8] = A[k].T
            consts[r:r + 8, 64 + r:64 + r + 8] = Bm[k].T
            for l, wire in enumerate((k, 4 + k, 8 + k)):
                consts[r:r + 8, 128 + c2 * 12 + wire] = S3[l]
    consts[:, 153] = 1.0
    return consts


def _build_program():
    import concourse.bass as bass
    from concourse import mybir
    from contextlib import ExitStack

    f32 = mybir.dt.float32
    f32r = mybir.dt.float32r
    AF = mybir.ActivationFunctionType

    # Skip the constructor's all-engine barrier (~2-3us of EVSEM handshake).
    # It only guards the const-AP memsets, and this kernel never reads a
    # const AP: every activation bias is passed explicitly from consts.
    orig_barrier = bass.Bass.all_engine_barrier
    bass.Bass.all_engine_barrier = lambda self, **kw: None
    try:
        nc = bass.Bass()
    finally:
        bass.Bass.all_engine_barrier = orig_barrier

    ang = nc.declare_dram_parameter("ang", [128, 26], f32, isOutput=False)
    consts = nc.declare_dram_parameter("consts", [128, CONST_COLS], f32, isOutput=False)
    evout = nc.declare_dram_parameter("evout", [24, 128], f32, isOutput=True)

    def mm(x):
        return x.bitcast(f32r) if USE_F32R else x

    with ExitStack() as ctx:
        e = ctx.enter_context
        const_sb = e(nc.sbuf_tensor([128, CONST_COLS], f32))
        ident = e(nc.sbuf_tensor([128, 128], f32))
        ang_t = e(nc.sbuf_tensor([128, 26], f32))
        u = e(nc.sbuf_tensor([128, 24], f32))
        usq = e(nc.sbuf_tensor([128, 24], f32))
        cs = e(nc.sbuf_tensor([128, 48], f32))   # col = c2*24 + j2*12 + w
        z4 = e(nc.sbuf_tensor([128, 32], f32))   # col = c2*16 + k*4 + j2*2 + j1
        m = e(nc.sbuf_tensor([128, 64], f32))    # col = c2*32 + k*8 + j2*4 + j1*2 + j0
        mT_sb = e(nc.sbuf_tensor([64, 128], f32))
        sq = e(nc.sbuf_tensor([64, 256], f32))
        ev_sb = e(nc.sbuf_tensor([24, 128], f32))
        scratch = e(nc.sbuf_tensor([1, 2], f32))
        mT_ps = e(nc.psum_tensor([64, 128], f32))
        reim_ps = e(nc.psum_tensor([64, 256], f32))
        ev_ps = e(nc.psum_tensor([24, 128], f32))
        s_dma = e(nc.semaphore("s_dma"))    # ang in + ev out
        s_dmac = e(nc.semaphore("s_dmac"))  # consts in
        s_act = e(nc.semaphore("s_act"))
        s_pe = e(nc.semaphore("s_pe"))
        s_dve = e(nc.semaphore("s_dve"))
        s_gp = e(nc.semaphore("s_gp"))
        bias0 = ang_t[:, 24:25]
        bias1 = ang_t[:, 25:26]
        block = e(nc.Block(no_gpsimd_drain=True))

        @block.sync
        def _(sync):
            sync.dma_start(out=ang_t[:], in_=ang[:, :]).then_inc(s_dma, 16)
            sync.dma_start(
                out=const_sb[:], in_=consts[:, :]).then_inc(s_dmac, 16)
            sync.wait_ge(s_act, 9)
            sync.dma_start(out=evout[:, :], in_=ev_sb[:]).then_inc(s_dma, 16)
            sync.wait_ge(s_dma, 32)
            sync.wait_ge(s_dmac, 16)

        @block.gpsimd
        def _(gpsimd):
            # transpose identity, built on the otherwise idle Pool engine
            nc.gpsimd.memset(ident[:], 0.0).then_inc(s_gp, 1)
            gpsimd.wait_ge(s_gp, 1)
            nc.gpsimd.affine_select(
                out=ident[:], in_=ident[:],
                compare_op=mybir.AluOpType.not_equal,
                fill=1.0, base=0, pattern=[[-1, 128]], channel_multiplier=1,
            ).then_inc(s_gp, 1)

        @block.scalar
        def _(scalar):
            cv = cs[:].rearrange("p (c j w) -> p c j w", c=2, j=2, w=12)
            a2 = ang_t[:, 0:24].rearrange("p (c w) -> p c w", c=2)
            # dummy op to pull the ACT function table in during the DMA wait
            nc.scalar.memzero(scratch[:]).then_inc(s_act, 1)                     # 1
            scalar.wait_ge(s_act, 1)
            nc.scalar.activation(scratch[:, 0:1], scratch[:, 1:2], AF.Sin,
                                 bias=scratch[:, 1:2], scale=0.25).then_inc(s_act, 1)  # 2
            scalar.wait_ge(s_dma, 16)
            # u = sin(x/4); sin(x/2) = Sin(0.5 x); cos(x/2) = 1 - 2u^2
            # (|x| < 2pi for N(0,1) angles, so all Sin operands are in-range)
            nc.scalar.activation(u[:], ang_t[:, 0:24], AF.Sin,
                                 bias=bias0, scale=0.25).then_inc(s_act, 1)      # 3
            nc.scalar.activation(cv[:, :, 1, :], a2, AF.Sin,
                                 bias=bias0, scale=0.5).then_inc(s_act, 1)       # 4
            scalar.wait_ge(s_act, 3)
            nc.scalar.activation(usq[:], u[:], AF.Square,
                                 bias=bias0).then_inc(s_act, 1)                  # 5
            scalar.wait_ge(s_act, 5)
            nc.scalar.activation(
                cv[:, :, 0, :], usq[:].rearrange("p (c w) -> p c w", c=2),
                AF.Identity, bias=bias1, scale=-2.0).then_inc(s_act, 1)          # 6
            scalar.wait_ge(s_pe, 1)
            nc.scalar.copy(mT_sb[:], mT_ps[:]).then_inc(s_act, 1)                # 7
            scalar.wait_ge(s_pe, 3)
            nc.scalar.activation(sq[:], reim_ps[:], AF.Square,
                                 bias=bias0[0:64]).then_inc(s_act, 1)            # 8
            scalar.wait_ge(s_pe, 5)
            nc.scalar.copy(ev_sb[:], ev_ps[:]).then_inc(s_act, 1)                # 9

        @block.vector
        def _(vector):
            # kron build: z4[(k,j2,j1)] = cs[j2*12+k] * cs[j1*12+4+k]
            #             m[(k,j2,j1,j0)] = z4[(k,j2,j1)] * cs[j0*12+8+k]
            csw = cs[:].rearrange("p (c j w) -> p c w j", c=2, j=2, w=12)
            z4v = z4[:].rearrange("p (c k a b) -> p c k a b", c=2, k=4, a=2, b=2)
            mv = m[:].rearrange("p (c k a b) -> p c k a b", c=2, k=4, a=4, b=2)
            vector.wait_ge(s_act, 6)
            for c2 in range(2):
                in0 = csw[:, c2, 0:4, :]                             # [p, k, j2]
                in0 = in0.unsqueeze(3).to_broadcast((128, 4, 2, 2))  # bcast j1
                in1 = csw[:, c2, 4:8, :]                             # [p, k, j1]
                in1 = in1.unsqueeze(2).to_broadcast((128, 4, 2, 2))  # bcast j2
                nc.vector.tensor_mul(z4v[:, c2], in0, in1).then_inc(s_dve, 1)
            vector.wait_ge(s_dve, 2)
            for c2 in range(2):
                in0 = z4v[:, c2].rearrange("p k a b -> p k (a b)")   # [p, 4, 4]
                in0 = in0.unsqueeze(3).to_broadcast((128, 4, 4, 2))  # bcast j0
                in1 = csw[:, c2, 8:12, :]                            # [p, k, j0]
                in1 = in1.unsqueeze(2).to_broadcast((128, 4, 4, 2))  # bcast j2 j1
                nc.vector.tensor_mul(mv[:, c2], in0, in1).then_inc(s_dve, 1)

        @block.tensor
        def _(tensor):
            tensor.wait_ge(s_gp, 2)
            tensor.wait_ge(s_dve, 4)
            nc.tensor.transpose(mT_ps[:], m[:], ident[:]).then_inc(s_pe, 1)
            tensor.wait_ge(s_dmac, 16)
            tensor.wait_ge(s_act, 7)
            nc.tensor.matmul(reim_ps[:, 0:128], mm(const_sb[0:64, 0:64]), mm(mT_sb[:]),
                             start=True, stop=True).then_inc(s_pe, 1)
            nc.tensor.matmul(reim_ps[:, 128:256], mm(const_sb[0:64, 64:128]), mm(mT_sb[:]),
                             start=True, stop=True).then_inc(s_pe, 1)
            tensor.wait_ge(s_act, 8)
            # ev = ST^T (sq_re + sq_im): two matmuls accumulating in PSUM
            nc.tensor.matmul(ev_ps[:], mm(const_sb[0:64, 128:152]), mm(sq[:, 0:128]),
                             start=True, stop=False).then_inc(s_pe, 1)
            nc.tensor.matmul(ev_ps[:], mm(const_sb[0:64, 128:152]), mm(sq[:, 128:256]),
                             start=False, stop=True).then_inc(s_pe, 1)

    return nc


_CACHE = {}
LAST_RESULTS = None


def _get_program():
    if "nc" not in _CACHE:
        _CACHE["nc"] = _build_program()
    return _CACHE["nc"]


def kernel(x, fqconv_weights):
    global LAST_RESULTS
    from concourse.bass_utils import run_bass_kernel_spmd

    x = np.ascontiguousarray(np.asarray(x, np.float32))
    consts = _build_consts(fqconv_weights)
    nc = _get_program()

    in_maps = []
    for i in range(N_CORES):
        g, b0 = i // 2, (i % 2) * PER_CORE
        ang_i = x[b0:b0 + PER_CORE, 3 * g:3 * g + 3].reshape(PER_CORE, 12)
        blob = np.empty((128, 26), np.float32)
        blob[:, 0:12] = ang_i[0::2]
        blob[:, 12:24] = ang_i[1::2]
        blob[:, 24] = 0.0
        blob[:, 25] = 1.0
        in_maps.append({"ang": blob, "consts": consts})

    res = run_bass_kernel_spmd(
        nc, in_maps, list(range(N_CORES)),
        trace=bool(int(os.environ.get("FQ_TRACE", "0"))),
    )
    LAST_RESULTS = res

    ev_all = np.empty((N_CIRC, 12), np.float32)
    for i in range(N_CORES):
        t = res.results[i]["evout"].reshape(2, 12, 128)
        ev_all[i * PER_CORE:(i + 1) * PER_CORE] = (
            t.transpose(2, 0, 1).reshape(PER_CORE, 12))
    return ev_all.reshape(-1).reshape(B, 12, 2, 2).astype(np.float32)


# revision 27
# speedup vs baseline: 1.1314x; 1.0298x over previous
"""Trainium2 Bass kernel for nn_fqconv_ansatz_layer.

Math: the 12-qubit circuit factorizes into 4 independent 3-qubit circuits
(triples {k, 4+k, 8+k}): the initial RX layer is a product state and every
controlled gate pairs wires within one triple. Per circuit and triple:
  ev3 = S3 @ |W_k' m|^2,  m = kron(u_k, u_{4+k}, u_{8+k}),  u_w = (cos th/2, sin th/2)
with W_k' = (product of the 6 controlled 8x8 gates) @ diag((-i)^popcount) a
weights-only 8x8 complex matrix precomputed on host.

Device kernel (per core, 256 circuits, raw Bass with explicit semaphores —
the CoreV3 ISA allows only one embedded sync wait per instruction, so waits
are standalone): cos/sin via Sin activations (quarter/half-angle keeps the
operand inside the valid [-pi, pi] range), the kron via 2 broadcast vector
multiplies, one PE transpose, then blockdiag matmuls + squares + a signed
marginal matmul pair accumulating in PSUM.
"""

import os
import numpy as np

N_Q = 12
GATES = [
    ('z', 0, 3, 11), ('x', 1, 2, 10), ('z', 2, 1, 9), ('x', 3, 0, 8),
    ('x', 4, 3, 11), ('z', 5, 2, 10), ('x', 6, 1, 9), ('z', 7, 0, 8),
    ('z', 8, 11, 7), ('x', 9, 10, 6), ('z', 10, 9, 5), ('x', 11, 8, 4),
    ('x', 12, 11, 7), ('z', 13, 10, 6), ('x', 14, 9, 5), ('z', 15, 8, 4),
    ('z', 16, 7, 3), ('x', 17, 6, 2), ('z', 18, 5, 1), ('x', 19, 4, 0),
    ('x', 20, 7, 3), ('z', 21, 6, 2), ('x', 22, 5, 1), ('z', 23, 4, 0),
]

N_CORES = 8
B = 512
G = 4
N_CIRC = B * G            # 2048
PER_CORE = N_CIRC // N_CORES  # 256

# consts layout: [128, 154] f32
#   rows 0:64,  cols   0: 64  -> AT  (lhsT of Re(W') matmul, block-diag over (c2, k))
#   rows 0:64,  cols  64:128  -> BT  (lhsT of Im(W') matmul)
#   rows 0:64,  cols 128:152  -> ST  (lhsT of signed marginal sum, [64, 24])
#   col 152 -> zeros, col 153 -> ones (activation bias operands)
CONST_COLS = 154
USE_F32R = False  # fp32r needs f32r-rounded producers; not worth it here
N_WARM = 8       # PE p-state warm-up matmuls issued during the DMA wait


def _rx2(theta):
    t = theta * 0.5
    return np.array([[np.cos(t), -1j * np.sin(t)],
                     [-1j * np.sin(t), np.cos(t)]], np.complex128)


def _rz2(theta):
    t = theta * 0.5
    return np.array([[np.exp(-1j * t), 0.0], [0.0, np.exp(1j * t)]], np.complex128)


def _ctrl_gate_3q(U2, pc, pt):
    M = np.zeros((8, 8), np.complex128)
    for i in range(8):
        for j in range(8):
            ic, jc = (i >> pc) & 1, (j >> pc) & 1
            it, jt = (i >> pt) & 1, (j >> pt) & 1
            mask = ~((1 << pc) | (1 << pt)) & 7
            if (i & mask) != (j & mask) or ic != jc:
                continue
            M[i, j] = (U2[it, jt] if ic == 1 else (1.0 if it == jt else 0.0))
    return M


def _build_consts(weights):
    """weights [24] f32 -> consts [128, 280] f32 (shared by all cores)."""
    w = np.asarray(weights, np.float64)
    popc = np.array([bin(j).count('1') for j in range(8)])
    A = np.zeros((4, 8, 8))
    Bm = np.zeros((4, 8, 8))
    for k in range(4):
        bitpos = {k: 2, 4 + k: 1, 8 + k: 0}
        W = np.eye(8, dtype=np.complex128)
        for kind, wi, c, t in GATES:
            if c % 4 != k:
                continue
            U2 = _rz2(w[wi]) if kind == 'z' else _rx2(w[wi])
            W = _ctrl_gate_3q(U2, bitpos[c], bitpos[t]) @ W
        Wp = W @ np.diag((-1j) ** popc)
        A[k] = Wp.real
        Bm[k] = Wp.imag
    S3 = np.zeros((3, 8))
    for l in range(3):
        for j in range(8):
            S3[l, j] = 1.0 if ((j >> (2 - l)) & 1) == 0 else -1.0

    consts = np.zeros((128, CONST_COLS), np.float32)
    for c2 in range(2):
        for k in range(4):
            r = c2 * 32 + k * 8
            # lhsT[j_in, j_out] = M[j_out, j_in]
            consts[r:r + 8, r:r + 8] = A[k].T
            consts[r:r + 8, 64 + r:64 + r + 8] = Bm[k].T
            for l, wire in enumerate((k, 4 + k, 8 + k)):
                consts[r:r + 8, 128 + c2 * 12 + wire] = S3[l]
    consts[:, 153] = 1.0
    return consts


def _build_program():
    import concourse.bass as bass
    from concourse import mybir
    from contextlib import ExitStack

    f32 = mybir.dt.float32
    f32r = mybir.dt.float32r
    AF = mybir.ActivationFunctionType

    # Skip the constructor's all-engine barrier (~2-3us of EVSEM handshake).
    # It only guards the const-AP memsets, and this kernel never reads a
    # const AP: every activation bias is passed explicitly from consts.
    orig_barrier = bass.Bass.all_engine_barrier
    bass.Bass.all_engine_barrier = lambda self, **kw: None
    try:
        nc = bass.Bass()
    finally:
        bass.Bass.all_engine_barrier = orig_barrier

    ang = nc.declare_dram_parameter("ang", [128, 26], f32, isOutput=False)
    consts = nc.declare_dram_parameter("consts", [128, CONST_COLS], f32, isOutput=False)
    evout = nc.declare_dram_parameter("evout", [24, 128], f32, isOutput=True)

    def mm(x):
        return x.bitcast(f32r) if USE_F32R else x

    with ExitStack() as ctx:
        e = ctx.enter_context
        const_sb = e(nc.sbuf_tensor([128, CONST_COLS], f32))
        ident = e(nc.sbuf_tensor([128, 128], f32))
        ang_t = e(nc.sbuf_tensor([128, 26], f32))
        u = e(nc.sbuf_tensor([128, 24], f32))
        usq = e(nc.sbuf_tensor([128, 24], f32))
        cs = e(nc.sbuf_tensor([128, 48], f32))   # col = c2*24 + j2*12 + w
        z4 = e(nc.sbuf_tensor([128, 32], f32))   # col = c2*16 + k*4 + j2*2 + j1
        m = e(nc.sbuf_tensor([128, 64], f32))    # col = c2*32 + k*8 + j2*4 + j1*2 + j0
        mT_sb = e(nc.sbuf_tensor([64, 128], f32))
        sq = e(nc.sbuf_tensor([64, 256], f32))
        ev_sb = e(nc.sbuf_tensor([24, 128], f32))
        scratch = e(nc.sbuf_tensor([1, 2], f32))
        warm_w = e(nc.sbuf_tensor([128, 128], f32))
        mT_ps = e(nc.psum_tensor([64, 128], f32))
        warm_ps = e(nc.psum_tensor([128, 128], f32))
        reim_ps = e(nc.psum_tensor([64, 256], f32))
        ev_ps = e(nc.psum_tensor([24, 128], f32))
        s_dma = e(nc.semaphore("s_dma"))    # ang in + ev out
        s_dmac = e(nc.semaphore("s_dmac"))  # consts in
        s_act = e(nc.semaphore("s_act"))
        s_pe = e(nc.semaphore("s_pe"))
        s_dve = e(nc.semaphore("s_dve"))
        s_gp = e(nc.semaphore("s_gp"))
        bias0 = ang_t[:, 24:25]
        bias1 = ang_t[:, 25:26]
        block = e(nc.Block(no_gpsimd_drain=True))

        @block.sync
        def _(sync):
            sync.wait_ge(s_dma, 32)
            sync.wait_ge(s_dmac, 16)

        @block.gpsimd
        def _(gpsimd):
            # transpose identity, built on the otherwise idle Pool engine
            nc.gpsimd.memset(ident[:], 0.0).then_inc(s_gp, 1)
            gpsimd.wait_ge(s_gp, 1)
            nc.gpsimd.affine_select(
                out=ident[:], in_=ident[:],
                compare_op=mybir.AluOpType.not_equal,
                fill=1.0, base=0, pattern=[[-1, 128]], channel_multiplier=1,
            ).then_inc(s_gp, 1)

        @block.scalar
        def _(scalar):
            cv = cs[:].rearrange("p (c j w) -> p c j w", c=2, j=2, w=12)
            a2 = ang_t[:, 0:24].rearrange("p (c w) -> p c w", c=2)
            # input DMAs triggered from the ACT queue, which starts executing
            # far earlier than the SP queue — the transfers overlap the
            # runtime's boot window
            scalar.dma_start(out=ang_t[:], in_=ang[:, :]).then_inc(s_dma, 16)
            scalar.dma_start(out=const_sb[:], in_=consts[:, :]).then_inc(s_dmac, 16)
            # dummy op to pull the ACT function table in during the DMA wait
            nc.scalar.memzero(scratch[:]).then_inc(s_act, 1)                     # 1
            scalar.wait_ge(s_act, 1)
            nc.scalar.activation(scratch[:, 0:1], scratch[:, 1:2], AF.Sin,
                                 bias=scratch[:, 1:2], scale=0.25).then_inc(s_act, 1)  # 2
            scalar.wait_ge(s_dma, 16)
            # u = sin(x/4); sin(x/2) = Sin(0.5 x); cos(x/2) = 1 - 2u^2
            # (|x| < 2pi for N(0,1) angles, so all Sin operands are in-range)
            nc.scalar.activation(u[:], ang_t[:, 0:24], AF.Sin,
                                 bias=bias0, scale=0.25).then_inc(s_act, 1)      # 3
            nc.scalar.activation(cv[:, :, 1, :], a2, AF.Sin,
                                 bias=bias0, scale=0.5).then_inc(s_act, 1)       # 4
            scalar.wait_ge(s_act, 3)
            nc.scalar.activation(usq[:], u[:], AF.Square,
                                 bias=bias0).then_inc(s_act, 1)                  # 5
            scalar.wait_ge(s_act, 5)
            nc.scalar.activation(
                cv[:, :, 0, :], usq[:].rearrange("p (c w) -> p c w", c=2),
                AF.Identity, bias=bias1, scale=-2.0).then_inc(s_act, 1)          # 6
            scalar.wait_ge(s_pe, 1)
            nc.scalar.copy(mT_sb[:], mT_ps[:]).then_inc(s_act, 1)                # 7
            scalar.wait_ge(s_pe, 3)
            nc.scalar.activation(sq[:], reim_ps[:], AF.Square,
                                 bias=bias0[0:64]).then_inc(s_act, 1)            # 8
            scalar.wait_ge(s_pe, 5)
            nc.scalar.copy(ev_sb[:], ev_ps[:]).then_inc(s_act, 1)                # 9
            scalar.wait_ge(s_act, 9)
            scalar.dma_start(out=evout[:, :], in_=ev_sb[:]).then_inc(s_dma, 16)

        @block.vector
        def _(vector):
            # kron build: z4[(k,j2,j1)] = cs[j2*12+k] * cs[j1*12+4+k]
            #             m[(k,j2,j1,j0)] = z4[(k,j2,j1)] * cs[j0*12+8+k]
            csw = cs[:].rearrange("p (c j w) -> p c w j", c=2, j=2, w=12)
            z4v = z4[:].rearrange("p (c k a b) -> p c k a b", c=2, k=4, a=2, b=2)
            mv = m[:].rearrange("p (c k a b) -> p c k a b", c=2, k=4, a=4, b=2)
            nc.vector.memset(warm_w[:], 0.0).then_inc(s_dve, 1)
            vector.wait_ge(s_act, 6)
            for c2 in range(2):
                in0 = csw[:, c2, 0:4, :]                             # [p, k, j2]
                in0 = in0.unsqueeze(3).to_broadcast((128, 4, 2, 2))  # bcast j1
                in1 = csw[:, c2, 4:8, :]                             # [p, k, j1]
                in1 = in1.unsqueeze(2).to_broadcast((128, 4, 2, 2))  # bcast j2
                nc.vector.tensor_mul(z4v[:, c2], in0, in1).then_inc(s_dve, 1)
            vector.wait_ge(s_dve, 3)
            for c2 in range(2):
                in0 = z4v[:, c2].rearrange("p k a b -> p k (a b)")   # [p, 4, 4]
                in0 = in0.unsqueeze(3).to_broadcast((128, 4, 4, 2))  # bcast j0
                in1 = csw[:, c2, 8:12, :]                            # [p, k, j0]
                in1 = in1.unsqueeze(2).to_broadcast((128, 4, 4, 2))  # bcast j2 j1
                nc.vector.tensor_mul(mv[:, c2], in0, in1).then_inc(s_dve, 1)

        @block.tensor
        def _(tensor):
            # dummy accumulation group to ramp the PE clock (HAM) while the
            # inputs are still in flight
            tensor.wait_ge(s_dve, 1)
            for i in range(N_WARM):
                nc.tensor.matmul(warm_ps[:], warm_w[:], warm_w[:],
                                 start=(i == 0), stop=(i == N_WARM - 1))
            tensor.wait_ge(s_gp, 2)
            tensor.wait_ge(s_dve, 5)
            nc.tensor.transpose(mT_ps[:], m[:], ident[:]).then_inc(s_pe, 1)
            tensor.wait_ge(s_dmac, 16)
            tensor.wait_ge(s_act, 7)
            nc.tensor.matmul(reim_ps[:, 0:128], mm(const_sb[0:64, 0:64]), mm(mT_sb[:]),
                             start=True, stop=True).then_inc(s_pe, 1)
            nc.tensor.matmul(reim_ps[:, 128:256], mm(const_sb[0:64, 64:128]), mm(mT_sb[:]),
                             start=True, stop=True).then_inc(s_pe, 1)
            tensor.wait_ge(s_act, 8)
            # ev = ST^T (sq_re + sq_im): two matmuls accumulating in PSUM
            nc.tensor.matmul(ev_ps[:], mm(const_sb[0:64, 128:152]), mm(sq[:, 0:128]),
                             start=True, stop=False).then_inc(s_pe, 1)
            nc.tensor.matmul(ev_ps[:], mm(const_sb[0:64, 128:152]), mm(sq[:, 128:256]),
                             start=False, stop=True).then_inc(s_pe, 1)

    return nc


_CACHE = {}
LAST_RESULTS = None


def _get_program():
    if "nc" not in _CACHE:
        _CACHE["nc"] = _build_program()
    return _CACHE["nc"]


def kernel(x, fqconv_weights):
    global LAST_RESULTS
    from concourse.bass_utils import run_bass_kernel_spmd

    x = np.ascontiguousarray(np.asarray(x, np.float32))
    consts = _build_consts(fqconv_weights)
    nc = _get_program()

    in_maps = []
    for i in range(N_CORES):
        g, b0 = i // 2, (i % 2) * PER_CORE
        ang_i = x[b0:b0 + PER_CORE, 3 * g:3 * g + 3].reshape(PER_CORE, 12)
        blob = np.empty((128, 26), np.float32)
        blob[:, 0:12] = ang_i[0::2]
        blob[:, 12:24] = ang_i[1::2]
        blob[:, 24] = 0.0
        blob[:, 25] = 1.0
        in_maps.append({"ang": blob, "consts": consts})

    res = run_bass_kernel_spmd(
        nc, in_maps, list(range(N_CORES)),
        trace=bool(int(os.environ.get("FQ_TRACE", "0"))),
    )
    LAST_RESULTS = res

    ev_all = np.empty((N_CIRC, 12), np.float32)
    for i in range(N_CORES):
        t = res.results[i]["evout"].reshape(2, 12, 128)
        ev_all[i * PER_CORE:(i + 1) * PER_CORE] = (
            t.transpose(2, 0, 1).reshape(PER_CORE, 12))
    return ev_all.reshape(-1).reshape(B, 12, 2, 2).astype(np.float32)
